# revision 11
# baseline (speedup 1.0000x reference)
"""MoE layer (top-2 of 8 experts) on 8 TRN2 NeuronCores, expert-parallel.

Sharding: expert parallelism. Core e holds expert e's weights. The host
computes routing (softmax + top-2) to build the dispatch: tokens routed to
expert e are gathered into a fixed-capacity buffer and sent to core e
(the single-host analogue of the all-to-all dispatch). Each core:
  - recomputes the router for its 1/8 shard of tokens to produce the
    aux-loss statistics (top-2 mask counts + prob sums) on device,
  - runs the expert SwiGLU (gate/up matmuls, silu*up, down matmul) over its
    dispatched tokens,
  - scales each token's output by its combine weight.
The host then scatter-adds the per-expert outputs back to token order
(the all-to-all combine) and finishes the aux loss reduction.

All heavy compute (3 big GEMMs, silu, router matmul/softmax/top-2 stats)
runs on device; the host only gathers/scatters and does O(E) reductions.
"""

import ml_dtypes
import numpy as np
from contextlib import ExitStack

import concourse.tile as tile
import concourse.mybir as mybir
from concourse import bacc
from concourse.bass_utils import run_bass_kernel_spmd

F32 = mybir.dt.float32
BF16 = mybir.dt.bfloat16
AF = mybir.ActivationFunctionType
ALU = mybir.AluOpType

B, S, H, I, E = 4, 1024, 1024, 2048, 8
T = B * S
TOP_K = 2
AUX_COEF = 0.01
NCORES = 8
TSH = T // NCORES          # router shard per core (tokens)
NB = 384                   # token-slot block size through the expert pipeline
KH = H // 128              # 8 contraction chunks over H
KI = I // 128              # 16 contraction chunks over I
MI = I // 128              # 16 output tiles over I (gate/up)

# matmul input dtype: float32r streams through the PE at full rate (1 cyc/row
# for moving dim >= 256) vs plain float32's 4 cyc/row; PSUM accum is fp32.
MM_DT = mybir.dt.float32r

_BUILD_CACHE = {}
LAST_RESULTS = None   # BassKernelResults of the most recent run (for test.py)


def _build(C):
    """Build + compile the per-core Bass program for capacity C (mult of NB)."""
    NBLK = C // NB
    CT = C // 128

    nc = bacc.Bacc(None, target_bir_lowering=False)

    xt_d = nc.dram_tensor("xt", [NBLK, 128, KH, NB], MM_DT, kind="ExternalInput")
    wg_d = nc.dram_tensor("wg", [MI, 128, KH, 128], MM_DT, kind="ExternalInput")
    wu_d = nc.dram_tensor("wu", [MI, 128, KH, 128], MM_DT, kind="ExternalInput")
    wd_d = nc.dram_tensor("wd", [128, KI, H], BF16, kind="ExternalInput")
    cw_d = nc.dram_tensor("cw", [128, CT], F32, kind="ExternalInput")
    xr_d = nc.dram_tensor("xr", [128, KH, TSH], F32, kind="ExternalInput")
    rw_d = nc.dram_tensor("rw", [128, KH, E], F32, kind="ExternalInput")
    y_d = nc.dram_tensor("y", [128, CT, H], F32, kind="ExternalOutput")
    aux_d = nc.dram_tensor("aux", [2 * E, 1], F32, kind="ExternalOutput")

    with tile.TileContext(nc) as tc, ExitStack() as ctx:
        const = ctx.enter_context(tc.tile_pool(name="const", bufs=1))
        xtp = ctx.enter_context(tc.tile_pool(name="xtp", bufs=1))
        htp = ctx.enter_context(tc.tile_pool(name="htp", bufs=1))
        wgp = ctx.enter_context(tc.tile_pool(name="wgp", bufs=5))
        wup = ctx.enter_context(tc.tile_pool(name="wup", bufs=5))
        wdp = ctx.enter_context(tc.tile_pool(name="wdp", bufs=1))
        sp = ctx.enter_context(tc.tile_pool(name="sp", bufs=3))
        yp = ctx.enter_context(tc.tile_pool(name="yp", bufs=4))
        rp = ctx.enter_context(tc.tile_pool(name="rp", bufs=2))
        ps = ctx.enter_context(tc.tile_pool(name="ps", bufs=8, space="PSUM"))

        # xt resident; block 0 + the first weight tiles go first so the PE
        # starts as early as possible
        xt_sb = xtp.tile([128, KH, C], MM_DT, tag="xt")
        wgus = []
        for m in range(MI):
            wg_sb = wgp.tile([128, KH, 128], MM_DT, tag="wg", name=f"wg{m}")
            wu_sb = wup.tile([128, KH, 128], MM_DT, tag="wu", name=f"wu{m}")
            if m == 0:
                kh2 = KH // 2
                nc.sync.dma_start(wg_sb[:, 0:kh2, :], wg_d[0, :, 0:kh2, :])
                nc.sync.dma_start(xt_sb[:, 0:kh2, 0:NB], xt_d[0][:, 0:kh2, :])
                nc.sync.dma_start(wu_sb[:, 0:kh2, :], wu_d[0, :, 0:kh2, :])
                nc.sync.dma_start(wg_sb[:, kh2:KH, :], wg_d[0, :, kh2:KH, :])
                nc.sync.dma_start(xt_sb[:, kh2:KH, 0:NB], xt_d[0][:, kh2:KH, :])
                nc.sync.dma_start(wu_sb[:, kh2:KH, :], wu_d[0, :, kh2:KH, :])
            else:
                nc.sync.dma_start(wg_sb[:], wg_d[m])
                nc.sync.dma_start(wu_sb[:], wu_d[m])
            wgus.append((wg_sb, wu_sb))
            if m == 2:
                for blk in range(1, NBLK):
                    nc.sync.dma_start(
                        xt_sb[:, :, blk * NB:(blk + 1) * NB],
                        xt_d[blk],
                    )
        cw_sb = const.tile([128, CT], F32)
        nc.sync.dma_start(cw_sb[:], cw_d[:])
        # wd is small in bf16: keep it resident, loaded during phase 1
        wd_sb = wdp.tile([128, KI, H], BF16, tag="wd")
        for i in range(KI):
            nc.sync.dma_start(wd_sb[:, i:i + 1, :], wd_d[:, i:i + 1, :])
        # prefetch router inputs early too (small)
        rw_sb = const.tile([128, KH, E], F32)
        nc.sync.dma_start(rw_sb[:], rw_d[:])
        xr_sb = const.tile([128, KH, TSH], F32)
        nc.sync.dma_start(xr_sb[:], xr_d[:])

        # ---- phase 1: gate/up + silu*up, weights loaded once (m-outer) ----
        ht = htp.tile([128, MI, C], BF16, tag="ht")
        for m in range(MI):
            wg_sb, wu_sb = wgus[m]
            for blk in range(NBLK):
                cs = slice(blk * NB, (blk + 1) * NB)
                psg = ps.tile([128, NB], F32, tag="ps", name=f"psg_{m}_{blk}")
                psu = ps.tile([128, NB], F32, tag="ps", name=f"psu_{m}_{blk}")
                for k in range(KH):
                    nc.tensor.matmul(
                        psg, wg_sb[:, k, :], xt_sb[:, k, cs],
                        start=(k == 0), stop=(k == KH - 1),
                    )
                for k in range(KH):
                    nc.tensor.matmul(
                        psu, wu_sb[:, k, :], xt_sb[:, k, cs],
                        start=(k == 0), stop=(k == KH - 1),
                    )
                s_sb = sp.tile([128, NB], F32, tag="s")
                nc.scalar.activation(s_sb, psg, AF.Silu)
                nc.vector.tensor_tensor(ht[:, m, cs], s_sb, psu, ALU.mult)

        # ---- router shard stats (logits + softmax stats), vectorized ----
        # (emitted between the phases: its vector work overlaps the down proj)
        ones_sb = const.tile([128, 1], F32)
        nc.any.memset(ones_sb[:], 1.0)
        NT = TSH // 128
        ps_r = ps.tile([128, NT, E], F32, tag="ps", name="psr")
        for t in range(NT):
            for k in range(KH):
                nc.tensor.matmul(
                    ps_r[:, t, :], xr_sb[:, k, t * 128:(t + 1) * 128],
                    rw_sb[:, k, :],
                    start=(t == 0 and k == 0), stop=(t == NT - 1 and k == KH - 1),
                    skip_group_check=True,
                )
        # logits ~ N(0,1): exp() cannot overflow fp32, skip the max-shift
        cat = rp.tile([128, NT, 2 * E], F32, tag="cat")
        ex = rp.tile([128, NT, E], F32, tag="ex")
        ssum = rp.tile([128, NT], F32, tag="ssum")
        rec = rp.tile([128, NT], F32, tag="rec")
        m1 = rp.tile([128, NT], F32, tag="m1")
        m2 = rp.tile([128, NT], F32, tag="m2")
        p2 = rp.tile([128, NT, E], F32, tag="p2")
        nc.scalar.activation(ex[:], ps_r, AF.Exp)
        nc.vector.tensor_reduce(ssum[:], ex[:], mybir.AxisListType.X, ALU.add)
        nc.vector.reciprocal(rec[:], ssum[:])
        nc.vector.tensor_tensor(
            cat[:, :, E:2 * E], ex[:],
            rec[:, :, None].to_broadcast((128, NT, E)), ALU.mult)
        nc.vector.tensor_reduce(m1[:], ex[:], mybir.AxisListType.X, ALU.max)
        nc.vector.tensor_tensor(
            p2[:], ex[:], m1[:, :, None].to_broadcast((128, NT, E)), ALU.is_equal)
        nc.vector.scalar_tensor_tensor(p2[:], p2[:], -1e6, ex[:], ALU.mult, ALU.add)
        nc.vector.tensor_reduce(m2[:], p2[:], mybir.AxisListType.X, ALU.max)
        nc.vector.tensor_tensor(
            cat[:, :, 0:E], ex[:],
            m2[:, :, None].to_broadcast((128, NT, E)), ALU.is_ge)

        # ---- phase 2: down proj, psum held over the full I contraction ----
        HS = H // 512
        for ct0 in range(0, CT, 2):
            nct = min(2, CT - ct0)
            psy = [
                [ps.tile([128, 512], F32, tag="ps", name=f"psy_{ct0}_{c}_{h}")
                 for h in range(HS)]
                for c in range(nct)
            ]
            for i in range(KI):
                for c in range(nct):
                    ct = ct0 + c
                    for hh in range(HS):
                        nc.tensor.matmul(
                            psy[c][hh],
                            ht[:, i, ct * 128:(ct + 1) * 128],
                            wd_sb[:, i, hh * 512:(hh + 1) * 512],
                            start=(i == 0), stop=(i == KI - 1),
                        )
            for c in range(nct):
                ct = ct0 + c
                for hh in range(HS):
                    y_sb = yp.tile([128, 512], F32, tag="y")
                    nc.vector.tensor_scalar_mul(
                        y_sb, psy[c][hh], cw_sb[:, ct:ct + 1]
                    )
                    nc.sync.dma_start(
                        y_d[:, ct, hh * 512:(hh + 1) * 512], y_sb
                    )

        # ---- tail: reduce the router stats across the token shard ----
        ps_aux = ps.tile([2 * E, 1], F32, tag="ps", name="psaux")
        for t in range(NT):
            nc.tensor.matmul(
                ps_aux, cat[:, t, :], ones_sb[:],
                start=(t == 0), stop=(t == NT - 1),
            )
        aux_sb = rp.tile([2 * E, 1], F32, tag="auxsb")
        nc.vector.tensor_copy(aux_sb[:], ps_aux)
        nc.sync.dma_start(aux_d[:], aux_sb[:])

    nc.compile()
    return nc


def _part_major(a, p=128):
    """[K*p, ...] -> [p, K, ...] with k = K_outer*p + p_inner on the first axis."""
    ko = a.shape[0] // p
    return np.ascontiguousarray(
        a.reshape(ko, p, *a.shape[1:]).transpose(1, 0, *range(2, a.ndim + 1))
    )


def kernel(hidden_states, router_w, w_gate, w_up, w_down):
    global LAST_RESULTS
    hidden_states = np.asarray(hidden_states, np.float32)
    router_w = np.ascontiguousarray(np.asarray(router_w, np.float32))
    w_gate = np.asarray(w_gate, np.float32)
    w_up = np.asarray(w_up, np.float32)
    w_down = np.asarray(w_down, np.float32)
    x = np.ascontiguousarray(hidden_states.reshape(T, H))

    # ---- host routing (for the dispatch indices + combine weights) ----
    logits = x @ router_w
    lmax = logits.max(-1, keepdims=True)
    p = np.exp(logits - lmax)
    p /= p.sum(-1, keepdims=True)
    rows = np.arange(T)
    i1 = p.argmax(-1)
    pm = p.copy()
    pm[rows, i1] = -1.0
    i2 = pm.argmax(-1)
    wsum = p[rows, i1] + p[rows, i2]

    idxs, cws = [], []
    for e in range(E):
        idx = np.nonzero((i1 == e) | (i2 == e))[0]
        idxs.append(idx)
        cws.append((p[idx, e] / wsum[idx]).astype(np.float32))
    maxn = max(len(ix) for ix in idxs)
    C = max(NB, ((maxn + NB - 1) // NB) * NB)
    CT = C // 128

    if C not in _BUILD_CACHE:
        _BUILD_CACHE[C] = _build(C)
    nc = _BUILD_CACHE[C]

    rw_t = _part_major(router_w)                      # [128, 8, 8]
    in_maps = []
    for e in range(E):
        idx = idxs[e]
        n = len(idx)
        xt = np.zeros((128, KH, C), np.float32)
        xt[:, :, :n] = _part_major(x[idx].T.copy())   # [128, 8, n]
        xt = np.ascontiguousarray(
            xt.reshape(128, KH, C // NB, NB).transpose(2, 0, 1, 3))
        cw = np.zeros(C, np.float32)
        cw[:n] = cws[e]
        cw = np.ascontiguousarray(cw.reshape(CT, 128).T)
        xr = _part_major(np.ascontiguousarray(x[e * TSH:(e + 1) * TSH].T))
        in_maps.append({
            "xt": xt,
            "wg": np.ascontiguousarray(
                w_gate[e].reshape(KH, 128, MI, 128).transpose(2, 1, 0, 3)),
            "wu": np.ascontiguousarray(
                w_up[e].reshape(KH, 128, MI, 128).transpose(2, 1, 0, 3)),
            "wd": _part_major(w_down[e].astype(ml_dtypes.bfloat16)),
            "cw": cw,
            "xr": xr,
            "rw": rw_t,
        })

    # trace defaults off; the test harness turns it on via BASS_TRACE=1
    res = run_bass_kernel_spmd(nc, in_maps, core_ids=list(range(NCORES)))
    LAST_RESULTS = res

    # ---- combine (scatter-add) + aux reduction ----
    out = np.zeros((T, H), np.float32)
    counts = np.zeros(E, np.float64)
    probsum = np.zeros(E, np.float64)
    for e in range(E):
        y = res.results[e]["y"]                       # [128, CT, H]
        y = y.transpose(1, 0, 2).reshape(C, H)
        idx = idxs[e]
        out[idx] += y[:len(idx)]
        aux = res.results[e]["aux"].reshape(2 * E)
        counts += aux[:E]
        probsum += aux[E:]
    aux_loss = np.float32(((counts / T) * (probsum / T)).sum() * E * AUX_COEF)
    return out.reshape(B, S, H), aux_loss



# revision 12
# speedup vs baseline: 1.0068x; 1.0068x over previous
"""MoE layer (top-2 of 8 experts) on 8 TRN2 NeuronCores, expert-parallel.

Sharding: expert parallelism. Core e holds expert e's weights. The host
computes routing (softmax + top-2) to build the dispatch: tokens routed to
expert e are gathered into a fixed-capacity buffer and sent to core e
(the single-host analogue of the all-to-all dispatch). Each core:
  - recomputes the router for its 1/8 shard of tokens to produce the
    aux-loss statistics (top-2 mask counts + prob sums) on device,
  - runs the expert SwiGLU (gate/up matmuls, silu*up, down matmul) over its
    dispatched tokens,
  - scales each token's output by its combine weight.
The host then scatter-adds the per-expert outputs back to token order
(the all-to-all combine) and finishes the aux loss reduction.

All heavy compute (3 big GEMMs, silu, router matmul/softmax/top-2 stats)
runs on device; the host only gathers/scatters and does O(E) reductions.
"""

import ml_dtypes
import numpy as np
from contextlib import ExitStack

import concourse.tile as tile
import concourse.mybir as mybir
from concourse import bacc
from concourse.bass_utils import run_bass_kernel_spmd

F32 = mybir.dt.float32
BF16 = mybir.dt.bfloat16
AF = mybir.ActivationFunctionType
ALU = mybir.AluOpType

B, S, H, I, E = 4, 1024, 1024, 2048, 8
T = B * S
TOP_K = 2
AUX_COEF = 0.01
NCORES = 8
TSH = T // NCORES          # router shard per core (tokens)
NB = 384                   # token-slot block size through the expert pipeline
KH = H // 128              # 8 contraction chunks over H
KI = I // 128              # 16 contraction chunks over I
MI = I // 128              # 16 output tiles over I (gate/up)

# matmul input dtype: float32r streams through the PE at full rate (1 cyc/row
# for moving dim >= 256) vs plain float32's 4 cyc/row; PSUM accum is fp32.
MM_DT = mybir.dt.float32r

_BUILD_CACHE = {}
LAST_RESULTS = None   # BassKernelResults of the most recent run (for test.py)


def _build(C):
    """Build + compile the per-core Bass program for capacity C (mult of NB)."""
    NBLK = C // NB
    CT = C // 128

    nc = bacc.Bacc(None, target_bir_lowering=False)

    xt_d = nc.dram_tensor("xt", [NBLK, 128, KH, NB], MM_DT, kind="ExternalInput")
    wg_d = nc.dram_tensor("wg", [MI, 128, KH, 128], MM_DT, kind="ExternalInput")
    wu_d = nc.dram_tensor("wu", [MI, 128, KH, 128], MM_DT, kind="ExternalInput")
    wd_d = nc.dram_tensor("wd", [128, KI, H], BF16, kind="ExternalInput")
    cw_d = nc.dram_tensor("cw", [128, CT], F32, kind="ExternalInput")
    xr_d = nc.dram_tensor("xr", [128, KH, TSH], F32, kind="ExternalInput")
    rw_d = nc.dram_tensor("rw", [128, KH, E], F32, kind="ExternalInput")
    y_d = nc.dram_tensor("y", [128, CT, H], F32, kind="ExternalOutput")
    aux_d = nc.dram_tensor("aux", [2 * E, 1], F32, kind="ExternalOutput")

    with tile.TileContext(nc) as tc, ExitStack() as ctx:
        const = ctx.enter_context(tc.tile_pool(name="const", bufs=1))
        xtp = ctx.enter_context(tc.tile_pool(name="xtp", bufs=1))
        htp = ctx.enter_context(tc.tile_pool(name="htp", bufs=1))
        wgp = ctx.enter_context(tc.tile_pool(name="wgp", bufs=5))
        wup = ctx.enter_context(tc.tile_pool(name="wup", bufs=5))
        wdp = ctx.enter_context(tc.tile_pool(name="wdp", bufs=1))
        sp = ctx.enter_context(tc.tile_pool(name="sp", bufs=3))
        yp = ctx.enter_context(tc.tile_pool(name="yp", bufs=4))
        rp = ctx.enter_context(tc.tile_pool(name="rp", bufs=2))
        ps = ctx.enter_context(tc.tile_pool(name="ps", bufs=8, space="PSUM"))

        # xt resident; block 0 + the first weight tiles go first so the PE
        # starts as early as possible
        xt_sb = xtp.tile([128, KH, C], MM_DT, tag="xt")
        wgus = []
        for m in range(MI):
            wg_sb = wgp.tile([128, KH, 128], MM_DT, tag="wg", name=f"wg{m}")
            wu_sb = wup.tile([128, KH, 128], MM_DT, tag="wu", name=f"wu{m}")
            if m == 0:
                kh2 = KH // 2
                nc.sync.dma_start(wg_sb[:, 0:kh2, :], wg_d[0, :, 0:kh2, :])
                nc.sync.dma_start(xt_sb[:, 0:kh2, 0:NB], xt_d[0][:, 0:kh2, :])
                nc.sync.dma_start(wg_sb[:, kh2:KH, :], wg_d[0, :, kh2:KH, :])
                nc.sync.dma_start(xt_sb[:, kh2:KH, 0:NB], xt_d[0][:, kh2:KH, :])
                nc.sync.dma_start(wu_sb[:, 0:kh2, :], wu_d[0, :, 0:kh2, :])
                nc.sync.dma_start(wu_sb[:, kh2:KH, :], wu_d[0, :, kh2:KH, :])
            else:
                nc.sync.dma_start(wg_sb[:], wg_d[m])
                nc.sync.dma_start(wu_sb[:], wu_d[m])
            wgus.append((wg_sb, wu_sb))
            if m == 0:
                for blk in range(1, NBLK):
                    nc.sync.dma_start(
                        xt_sb[:, :, blk * NB:(blk + 1) * NB],
                        xt_d[blk],
                    )
        cw_sb = const.tile([128, CT], F32)
        nc.sync.dma_start(cw_sb[:], cw_d[:])
        # wd is small in bf16: keep it resident, loaded during phase 1
        wd_sb = wdp.tile([128, KI, H], BF16, tag="wd")
        for i in range(KI):
            nc.sync.dma_start(wd_sb[:, i:i + 1, :], wd_d[:, i:i + 1, :])
        # prefetch router inputs early too (small)
        rw_sb = const.tile([128, KH, E], F32)
        nc.sync.dma_start(rw_sb[:], rw_d[:])
        xr_sb = const.tile([128, KH, TSH], F32)
        nc.sync.dma_start(xr_sb[:], xr_d[:])

        # ---- phase 1: gate/up + silu*up, weights loaded once (m-outer) ----
        ht = htp.tile([128, MI, C], BF16, tag="ht")
        for m in range(MI):
            wg_sb, wu_sb = wgus[m]
            for blk in range(NBLK):
                cs = slice(blk * NB, (blk + 1) * NB)
                psg = ps.tile([128, NB], F32, tag="ps", name=f"psg_{m}_{blk}")
                psu = ps.tile([128, NB], F32, tag="ps", name=f"psu_{m}_{blk}")
                for k in range(KH):
                    nc.tensor.matmul(
                        psg, wg_sb[:, k, :], xt_sb[:, k, cs],
                        start=(k == 0), stop=(k == KH - 1),
                    )
                for k in range(KH):
                    nc.tensor.matmul(
                        psu, wu_sb[:, k, :], xt_sb[:, k, cs],
                        start=(k == 0), stop=(k == KH - 1),
                    )
                s_sb = sp.tile([128, NB], F32, tag="s")
                nc.scalar.activation(s_sb, psg, AF.Silu)
                nc.vector.tensor_tensor(ht[:, m, cs], s_sb, psu, ALU.mult)

        # ---- router shard stats (logits + softmax stats), vectorized ----
        # (emitted between the phases: its vector work overlaps the down proj)
        ones_sb = const.tile([128, 1], F32)
        nc.any.memset(ones_sb[:], 1.0)
        NT = TSH // 128
        ps_r = ps.tile([128, NT, E], F32, tag="ps", name="psr")
        for t in range(NT):
            for k in range(KH):
                nc.tensor.matmul(
                    ps_r[:, t, :], xr_sb[:, k, t * 128:(t + 1) * 128],
                    rw_sb[:, k, :],
                    start=(t == 0 and k == 0), stop=(t == NT - 1 and k == KH - 1),
                    skip_group_check=True,
                )
        # logits ~ N(0,1): exp() cannot overflow fp32, skip the max-shift
        cat = rp.tile([128, NT, 2 * E], F32, tag="cat")
        ex = rp.tile([128, NT, E], F32, tag="ex")
        ssum = rp.tile([128, NT], F32, tag="ssum")
        rec = rp.tile([128, NT], F32, tag="rec")
        m1 = rp.tile([128, NT], F32, tag="m1")
        m2 = rp.tile([128, NT], F32, tag="m2")
        p2 = rp.tile([128, NT, E], F32, tag="p2")
        nc.scalar.activation(ex[:], ps_r, AF.Exp)
        nc.vector.tensor_reduce(ssum[:], ex[:], mybir.AxisListType.X, ALU.add)
        nc.vector.reciprocal(rec[:], ssum[:])
        nc.vector.tensor_tensor(
            cat[:, :, E:2 * E], ex[:],
            rec[:, :, None].to_broadcast((128, NT, E)), ALU.mult)
        nc.vector.tensor_reduce(m1[:], ex[:], mybir.AxisListType.X, ALU.max)
        nc.vector.tensor_tensor(
            p2[:], ex[:], m1[:, :, None].to_broadcast((128, NT, E)), ALU.is_equal)
        nc.vector.scalar_tensor_tensor(p2[:], p2[:], -1e6, ex[:], ALU.mult, ALU.add)
        nc.vector.tensor_reduce(m2[:], p2[:], mybir.AxisListType.X, ALU.max)
        nc.vector.tensor_tensor(
            cat[:, :, 0:E], ex[:],
            m2[:, :, None].to_broadcast((128, NT, E)), ALU.is_ge)

        # ---- phase 2: down proj, psum held over the full I contraction ----
        HS = H // 512
        for ct0 in range(0, CT, 2):
            nct = min(2, CT - ct0)
            psy = [
                [ps.tile([128, 512], F32, tag="ps", name=f"psy_{ct0}_{c}_{h}")
                 for h in range(HS)]
                for c in range(nct)
            ]
            for i in range(KI):
                for c in range(nct):
                    ct = ct0 + c
                    for hh in range(HS):
                        nc.tensor.matmul(
                            psy[c][hh],
                            ht[:, i, ct * 128:(ct + 1) * 128],
                            wd_sb[:, i, hh * 512:(hh + 1) * 512],
                            start=(i == 0), stop=(i == KI - 1),
                        )
            for c in range(nct):
                ct = ct0 + c
                for hh in range(HS):
                    y_sb = yp.tile([128, 512], F32, tag="y")
                    nc.vector.tensor_scalar_mul(
                        y_sb, psy[c][hh], cw_sb[:, ct:ct + 1]
                    )
                    nc.sync.dma_start(
                        y_d[:, ct, hh * 512:(hh + 1) * 512], y_sb
                    )

        # ---- tail: reduce the router stats across the token shard ----
        ps_aux = ps.tile([2 * E, 1], F32, tag="ps", name="psaux")
        for t in range(NT):
            nc.tensor.matmul(
                ps_aux, cat[:, t, :], ones_sb[:],
                start=(t == 0), stop=(t == NT - 1),
            )
        aux_sb = rp.tile([2 * E, 1], F32, tag="auxsb")
        nc.vector.tensor_copy(aux_sb[:], ps_aux)
        nc.sync.dma_start(aux_d[:], aux_sb[:])

    nc.compile()
    return nc


def _part_major(a, p=128):
    """[K*p, ...] -> [p, K, ...] with k = K_outer*p + p_inner on the first axis."""
    ko = a.shape[0] // p
    return np.ascontiguousarray(
        a.reshape(ko, p, *a.shape[1:]).transpose(1, 0, *range(2, a.ndim + 1))
    )


def kernel(hidden_states, router_w, w_gate, w_up, w_down):
    global LAST_RESULTS
    hidden_states = np.asarray(hidden_states, np.float32)
    router_w = np.ascontiguousarray(np.asarray(router_w, np.float32))
    w_gate = np.asarray(w_gate, np.float32)
    w_up = np.asarray(w_up, np.float32)
    w_down = np.asarray(w_down, np.float32)
    x = np.ascontiguousarray(hidden_states.reshape(T, H))

    # ---- host routing (for the dispatch indices + combine weights) ----
    logits = x @ router_w
    lmax = logits.max(-1, keepdims=True)
    p = np.exp(logits - lmax)
    p /= p.sum(-1, keepdims=True)
    rows = np.arange(T)
    i1 = p.argmax(-1)
    pm = p.copy()
    pm[rows, i1] = -1.0
    i2 = pm.argmax(-1)
    wsum = p[rows, i1] + p[rows, i2]

    idxs, cws = [], []
    for e in range(E):
        idx = np.nonzero((i1 == e) | (i2 == e))[0]
        idxs.append(idx)
        cws.append((p[idx, e] / wsum[idx]).astype(np.float32))
    maxn = max(len(ix) for ix in idxs)
    C = max(NB, ((maxn + NB - 1) // NB) * NB)
    CT = C // 128

    if C not in _BUILD_CACHE:
        _BUILD_CACHE[C] = _build(C)
    nc = _BUILD_CACHE[C]

    rw_t = _part_major(router_w)                      # [128, 8, 8]
    in_maps = []
    for e in range(E):
        idx = idxs[e]
        n = len(idx)
        xt = np.zeros((128, KH, C), np.float32)
        xt[:, :, :n] = _part_major(x[idx].T.copy())   # [128, 8, n]
        xt = np.ascontiguousarray(
            xt.reshape(128, KH, C // NB, NB).transpose(2, 0, 1, 3))
        cw = np.zeros(C, np.float32)
        cw[:n] = cws[e]
        cw = np.ascontiguousarray(cw.reshape(CT, 128).T)
        xr = _part_major(np.ascontiguousarray(x[e * TSH:(e + 1) * TSH].T))
        in_maps.append({
            "xt": xt,
            "wg": np.ascontiguousarray(
                w_gate[e].reshape(KH, 128, MI, 128).transpose(2, 1, 0, 3)),
            "wu": np.ascontiguousarray(
                w_up[e].reshape(KH, 128, MI, 128).transpose(2, 1, 0, 3)),
            "wd": _part_major(w_down[e].astype(ml_dtypes.bfloat16)),
            "cw": cw,
            "xr": xr,
            "rw": rw_t,
        })

    # trace defaults off; the test harness turns it on via BASS_TRACE=1
    res = run_bass_kernel_spmd(nc, in_maps, core_ids=list(range(NCORES)))
    LAST_RESULTS = res

    # ---- combine (scatter-add) + aux reduction ----
    out = np.zeros((T, H), np.float32)
    counts = np.zeros(E, np.float64)
    probsum = np.zeros(E, np.float64)
    for e in range(E):
        y = res.results[e]["y"]                       # [128, CT, H]
        y = y.transpose(1, 0, 2).reshape(C, H)
        idx = idxs[e]
        out[idx] += y[:len(idx)]
        aux = res.results[e]["aux"].reshape(2 * E)
        counts += aux[:E]
        probsum += aux[E:]
    aux_loss = np.float32(((counts / T) * (probsum / T)).sum() * E * AUX_COEF)
    return out.reshape(B, S, H), aux_loss



# revision 13
# speedup vs baseline: 1.0168x; 1.0099x over previous
"""MoE layer (top-2 of 8 experts) on 8 TRN2 NeuronCores, expert-parallel.

Sharding: expert parallelism. Core e holds expert e's weights. The host
computes routing (softmax + top-2) to build the dispatch: tokens routed to
expert e are gathered into a fixed-capacity buffer and sent to core e
(the single-host analogue of the all-to-all dispatch). Each core:
  - recomputes the router for its 1/8 shard of tokens to produce the
    aux-loss statistics (top-2 mask counts + prob sums) on device,
  - runs the expert SwiGLU (gate/up matmuls, silu*up, down matmul) over its
    dispatched tokens,
  - scales each token's output by its combine weight.
The host then scatter-adds the per-expert outputs back to token order
(the all-to-all combine) and finishes the aux loss reduction.

All heavy compute (3 big GEMMs, silu, router matmul/softmax/top-2 stats)
runs on device; the host only gathers/scatters and does O(E) reductions.
"""

import ml_dtypes
import numpy as np
from contextlib import ExitStack

import concourse.tile as tile
import concourse.mybir as mybir
from concourse import bacc
from concourse.bass_utils import run_bass_kernel_spmd

F32 = mybir.dt.float32
BF16 = mybir.dt.bfloat16
AF = mybir.ActivationFunctionType
ALU = mybir.AluOpType

B, S, H, I, E = 4, 1024, 1024, 2048, 8
T = B * S
TOP_K = 2
AUX_COEF = 0.01
NCORES = 8
TSH = T // NCORES          # router shard per core (tokens)
NB = 384                   # token-slot block size through the expert pipeline
KH = H // 128              # 8 contraction chunks over H
KI = I // 128              # 16 contraction chunks over I
MI = I // 128              # 16 output tiles over I (gate/up)

# matmul input dtype: float32r streams through the PE at full rate (1 cyc/row
# for moving dim >= 256) vs plain float32's 4 cyc/row; PSUM accum is fp32.
MM_DT = mybir.dt.float32r

_BUILD_CACHE = {}
LAST_RESULTS = None   # BassKernelResults of the most recent run (for test.py)


def _build(C):
    """Build + compile the per-core Bass program for capacity C (mult of NB)."""
    NBLK = C // NB
    CT = C // 128

    nc = bacc.Bacc(None, target_bir_lowering=False)

    xt_d = nc.dram_tensor("xt", [NBLK, 128, KH, NB], MM_DT, kind="ExternalInput")
    wg_d = nc.dram_tensor("wg", [MI, 128, KH, 128], MM_DT, kind="ExternalInput")
    wu_d = nc.dram_tensor("wu", [MI, 128, KH, 128], MM_DT, kind="ExternalInput")
    wd_d = nc.dram_tensor("wd", [128, KI, H], BF16, kind="ExternalInput")
    cw_d = nc.dram_tensor("cw", [128, CT], F32, kind="ExternalInput")
    xr_d = nc.dram_tensor("xr", [128, KH, TSH], F32, kind="ExternalInput")
    rw_d = nc.dram_tensor("rw", [128, KH, E], F32, kind="ExternalInput")
    y_d = nc.dram_tensor("y", [128, CT, H], F32, kind="ExternalOutput")
    aux_d = nc.dram_tensor("aux", [2 * E, 1], F32, kind="ExternalOutput")

    with tile.TileContext(nc) as tc, ExitStack() as ctx:
        const = ctx.enter_context(tc.tile_pool(name="const", bufs=1))
        xtp = ctx.enter_context(tc.tile_pool(name="xtp", bufs=1))
        htp = ctx.enter_context(tc.tile_pool(name="htp", bufs=1))
        wgp = ctx.enter_context(tc.tile_pool(name="wgp", bufs=8))
        wup = ctx.enter_context(tc.tile_pool(name="wup", bufs=8))
        wdp = ctx.enter_context(tc.tile_pool(name="wdp", bufs=1))
        sp = ctx.enter_context(tc.tile_pool(name="sp", bufs=4))
        yp = ctx.enter_context(tc.tile_pool(name="yp", bufs=4))
        rp = ctx.enter_context(tc.tile_pool(name="rp", bufs=2))
        ps = ctx.enter_context(tc.tile_pool(name="ps", bufs=8, space="PSUM"))

        # xt resident; block 0 + the first weight tiles go first so the PE
        # starts as early as possible
        xt_sb = xtp.tile([128, KH, C], MM_DT, tag="xt")
        wgus = []
        for m in range(MI):
            wg_sb = wgp.tile([128, KH, 128], MM_DT, tag="wg", name=f"wg{m}")
            wu_sb = wup.tile([128, KH, 128], MM_DT, tag="wu", name=f"wu{m}")
            if m == 0:
                kh2 = KH // 2
                nc.sync.dma_start(wg_sb[:, 0:kh2, :], wg_d[0, :, 0:kh2, :])
                nc.sync.dma_start(xt_sb[:, 0:kh2, 0:NB], xt_d[0][:, 0:kh2, :])
                nc.sync.dma_start(wg_sb[:, kh2:KH, :], wg_d[0, :, kh2:KH, :])
                nc.sync.dma_start(xt_sb[:, kh2:KH, 0:NB], xt_d[0][:, kh2:KH, :])
                nc.sync.dma_start(wu_sb[:, 0:kh2, :], wu_d[0, :, 0:kh2, :])
                nc.sync.dma_start(wu_sb[:, kh2:KH, :], wu_d[0, :, kh2:KH, :])
            else:
                nc.sync.dma_start(wg_sb[:], wg_d[m])
                nc.sync.dma_start(wu_sb[:], wu_d[m])
            wgus.append((wg_sb, wu_sb))
            if m == 0:
                for blk in range(1, NBLK):
                    nc.sync.dma_start(
                        xt_sb[:, :, blk * NB:(blk + 1) * NB],
                        xt_d[blk],
                    )
        cw_sb = const.tile([128, CT], F32)
        nc.sync.dma_start(cw_sb[:], cw_d[:])
        # wd is small in bf16: keep it resident, loaded during phase 1
        wd_sb = wdp.tile([128, KI, H], BF16, tag="wd")
        for i in range(KI):
            nc.sync.dma_start(wd_sb[:, i:i + 1, :], wd_d[:, i:i + 1, :])
        # prefetch router inputs early too (small)
        rw_sb = const.tile([128, KH, E], F32)
        nc.sync.dma_start(rw_sb[:], rw_d[:])
        xr_sb = const.tile([128, KH, TSH], F32)
        nc.sync.dma_start(xr_sb[:], xr_d[:])

        # ---- phase 1: gate/up + silu*up, weights loaded once (m-outer) ----
        ht = htp.tile([128, MI, C], BF16, tag="ht")
        for m in range(MI):
            wg_sb, wu_sb = wgus[m]
            for blk in range(NBLK):
                cs = slice(blk * NB, (blk + 1) * NB)
                psg = ps.tile([128, NB], F32, tag="ps", name=f"psg_{m}_{blk}")
                psu = ps.tile([128, NB], F32, tag="ps", name=f"psu_{m}_{blk}")
                for k in range(KH):
                    nc.tensor.matmul(
                        psg, wg_sb[:, k, :], xt_sb[:, k, cs],
                        start=(k == 0), stop=(k == KH - 1),
                    )
                for k in range(KH):
                    nc.tensor.matmul(
                        psu, wu_sb[:, k, :], xt_sb[:, k, cs],
                        start=(k == 0), stop=(k == KH - 1),
                    )
                s_sb = sp.tile([128, NB], F32, tag="s")
                nc.scalar.activation(s_sb, psg, AF.Silu)
                nc.vector.tensor_tensor(ht[:, m, cs], s_sb, psu, ALU.mult)

        # ---- router shard stats (logits + softmax stats), vectorized ----
        # (emitted between the phases: its vector work overlaps the down proj)
        ones_sb = const.tile([128, 1], F32)
        nc.any.memset(ones_sb[:], 1.0)
        NT = TSH // 128
        ps_r = ps.tile([128, NT, E], F32, tag="ps", name="psr")
        for t in range(NT):
            for k in range(KH):
                nc.tensor.matmul(
                    ps_r[:, t, :], xr_sb[:, k, t * 128:(t + 1) * 128],
                    rw_sb[:, k, :],
                    start=(t == 0 and k == 0), stop=(t == NT - 1 and k == KH - 1),
                    skip_group_check=True,
                )
        # logits ~ N(0,1): exp() cannot overflow fp32, skip the max-shift
        cat = rp.tile([128, NT, 2 * E], F32, tag="cat")
        ex = rp.tile([128, NT, E], F32, tag="ex")
        ssum = rp.tile([128, NT], F32, tag="ssum")
        rec = rp.tile([128, NT], F32, tag="rec")
        m1 = rp.tile([128, NT], F32, tag="m1")
        m2 = rp.tile([128, NT], F32, tag="m2")
        p2 = rp.tile([128, NT, E], F32, tag="p2")
        nc.scalar.activation(ex[:], ps_r, AF.Exp)
        nc.vector.tensor_reduce(ssum[:], ex[:], mybir.AxisListType.X, ALU.add)
        nc.vector.reciprocal(rec[:], ssum[:])
        nc.vector.tensor_tensor(
            cat[:, :, E:2 * E], ex[:],
            rec[:, :, None].to_broadcast((128, NT, E)), ALU.mult)
        nc.vector.tensor_reduce(m1[:], ex[:], mybir.AxisListType.X, ALU.max)
        nc.vector.tensor_tensor(
            p2[:], ex[:], m1[:, :, None].to_broadcast((128, NT, E)), ALU.is_equal)
        nc.vector.scalar_tensor_tensor(p2[:], p2[:], -1e6, ex[:], ALU.mult, ALU.add)
        nc.vector.tensor_reduce(m2[:], p2[:], mybir.AxisListType.X, ALU.max)
        nc.vector.tensor_tensor(
            cat[:, :, 0:E], ex[:],
            m2[:, :, None].to_broadcast((128, NT, E)), ALU.is_ge)

        # ---- phase 2: down proj, psum held over the full I contraction ----
        HS = H // 512
        for ct0 in range(0, CT, 2):
            nct = min(2, CT - ct0)
            psy = [
                [ps.tile([128, 512], F32, tag="ps", name=f"psy_{ct0}_{c}_{h}")
                 for h in range(HS)]
                for c in range(nct)
            ]
            for i in range(KI):
                for c in range(nct):
                    ct = ct0 + c
                    for hh in range(HS):
                        nc.tensor.matmul(
                            psy[c][hh],
                            ht[:, i, ct * 128:(ct + 1) * 128],
                            wd_sb[:, i, hh * 512:(hh + 1) * 512],
                            start=(i == 0), stop=(i == KI - 1),
                        )
            for c in range(nct):
                ct = ct0 + c
                for hh in range(HS):
                    y_sb = yp.tile([128, 512], F32, tag="y")
                    nc.vector.tensor_scalar_mul(
                        y_sb, psy[c][hh], cw_sb[:, ct:ct + 1]
                    )
                    nc.sync.dma_start(
                        y_d[:, ct, hh * 512:(hh + 1) * 512], y_sb
                    )

        # ---- tail: reduce the router stats across the token shard ----
        ps_aux = ps.tile([2 * E, 1], F32, tag="ps", name="psaux")
        for t in range(NT):
            nc.tensor.matmul(
                ps_aux, cat[:, t, :], ones_sb[:],
                start=(t == 0), stop=(t == NT - 1),
            )
        aux_sb = rp.tile([2 * E, 1], F32, tag="auxsb")
        nc.vector.tensor_copy(aux_sb[:], ps_aux)
        nc.sync.dma_start(aux_d[:], aux_sb[:])

    nc.compile()
    return nc


def _part_major(a, p=128):
    """[K*p, ...] -> [p, K, ...] with k = K_outer*p + p_inner on the first axis."""
    ko = a.shape[0] // p
    return np.ascontiguousarray(
        a.reshape(ko, p, *a.shape[1:]).transpose(1, 0, *range(2, a.ndim + 1))
    )


def kernel(hidden_states, router_w, w_gate, w_up, w_down):
    global LAST_RESULTS
    hidden_states = np.asarray(hidden_states, np.float32)
    router_w = np.ascontiguousarray(np.asarray(router_w, np.float32))
    w_gate = np.asarray(w_gate, np.float32)
    w_up = np.asarray(w_up, np.float32)
    w_down = np.asarray(w_down, np.float32)
    x = np.ascontiguousarray(hidden_states.reshape(T, H))

    # ---- host routing (for the dispatch indices + combine weights) ----
    logits = x @ router_w
    lmax = logits.max(-1, keepdims=True)
    p = np.exp(logits - lmax)
    p /= p.sum(-1, keepdims=True)
    rows = np.arange(T)
    i1 = p.argmax(-1)
    pm = p.copy()
    pm[rows, i1] = -1.0
    i2 = pm.argmax(-1)
    wsum = p[rows, i1] + p[rows, i2]

    idxs, cws = [], []
    for e in range(E):
        idx = np.nonzero((i1 == e) | (i2 == e))[0]
        idxs.append(idx)
        cws.append((p[idx, e] / wsum[idx]).astype(np.float32))
    maxn = max(len(ix) for ix in idxs)
    C = max(NB, ((maxn + NB - 1) // NB) * NB)
    CT = C // 128

    if C not in _BUILD_CACHE:
        _BUILD_CACHE[C] = _build(C)
    nc = _BUILD_CACHE[C]

    rw_t = _part_major(router_w)                      # [128, 8, 8]
    in_maps = []
    for e in range(E):
        idx = idxs[e]
        n = len(idx)
        xt = np.zeros((128, KH, C), np.float32)
        xt[:, :, :n] = _part_major(x[idx].T.copy())   # [128, 8, n]
        xt = np.ascontiguousarray(
            xt.reshape(128, KH, C // NB, NB).transpose(2, 0, 1, 3))
        cw = np.zeros(C, np.float32)
        cw[:n] = cws[e]
        cw = np.ascontiguousarray(cw.reshape(CT, 128).T)
        xr = _part_major(np.ascontiguousarray(x[e * TSH:(e + 1) * TSH].T))
        in_maps.append({
            "xt": xt,
            "wg": np.ascontiguousarray(
                w_gate[e].reshape(KH, 128, MI, 128).transpose(2, 1, 0, 3)),
            "wu": np.ascontiguousarray(
                w_up[e].reshape(KH, 128, MI, 128).transpose(2, 1, 0, 3)),
            "wd": _part_major(w_down[e].astype(ml_dtypes.bfloat16)),
            "cw": cw,
            "xr": xr,
            "rw": rw_t,
        })

    # trace defaults off; the test harness turns it on via BASS_TRACE=1
    res = run_bass_kernel_spmd(nc, in_maps, core_ids=list(range(NCORES)))
    LAST_RESULTS = res

    # ---- combine (scatter-add) + aux reduction ----
    out = np.zeros((T, H), np.float32)
    counts = np.zeros(E, np.float64)
    probsum = np.zeros(E, np.float64)
    for e in range(E):
        y = res.results[e]["y"]                       # [128, CT, H]
        y = y.transpose(1, 0, 2).reshape(C, H)
        idx = idxs[e]
        out[idx] += y[:len(idx)]
        aux = res.results[e]["aux"].reshape(2 * E)
        counts += aux[:E]
        probsum += aux[E:]
    aux_loss = np.float32(((counts / T) * (probsum / T)).sum() * E * AUX_COEF)
    return out.reshape(B, S, H), aux_loss



# revision 14
# speedup vs baseline: 1.0490x; 1.0317x over previous
"""MoE layer (top-2 of 8 experts) on 8 TRN2 NeuronCores, expert-parallel.

Sharding: expert parallelism. Core e holds expert e's weights. The host
computes routing (softmax + top-2) to build the dispatch: tokens routed to
expert e are gathered into a fixed-capacity buffer and sent to core e
(the single-host analogue of the all-to-all dispatch). Each core:
  - recomputes the router for its 1/8 shard of tokens to produce the
    aux-loss statistics (top-2 mask counts + prob sums) on device,
  - runs the expert SwiGLU (gate/up matmuls, silu*up, down matmul) over its
    dispatched tokens,
  - scales each token's output by its combine weight.
The host then scatter-adds the per-expert outputs back to token order
(the all-to-all combine) and finishes the aux loss reduction.

All heavy compute (3 big GEMMs, silu, router matmul/softmax/top-2 stats)
runs on device; the host only gathers/scatters and does O(E) reductions.
"""

import ml_dtypes
import numpy as np
from contextlib import ExitStack

import concourse.tile as tile
import concourse.mybir as mybir
from concourse import bacc
from concourse.bass_utils import run_bass_kernel_spmd

F32 = mybir.dt.float32
BF16 = mybir.dt.bfloat16
AF = mybir.ActivationFunctionType
ALU = mybir.AluOpType

B, S, H, I, E = 4, 1024, 1024, 2048, 8
T = B * S
TOP_K = 2
AUX_COEF = 0.01
NCORES = 8
TSH = T // NCORES          # router shard per core (tokens)
NB = 384                   # token-slot block size through the expert pipeline
KH = H // 128              # 8 contraction chunks over H
KI = I // 128              # 16 contraction chunks over I
MI = I // 128              # 16 output tiles over I (gate/up)

# matmul input dtype: float32r streams through the PE at full rate (1 cyc/row
# for moving dim >= 256) vs plain float32's 4 cyc/row; PSUM accum is fp32.
MM_DT = mybir.dt.float32r

_BUILD_CACHE = {}
LAST_RESULTS = None   # BassKernelResults of the most recent run (for test.py)


def _build(C):
    """Build + compile the per-core Bass program for capacity C (mult of NB)."""
    NBLK = C // NB
    CT = C // 128

    nc = bacc.Bacc(None, target_bir_lowering=False)

    xt_d = nc.dram_tensor("xt", [NBLK, 128, KH, NB], MM_DT, kind="ExternalInput")
    wg_d = nc.dram_tensor("wg", [MI, 128, KH, 128], MM_DT, kind="ExternalInput")
    wu_d = nc.dram_tensor("wu", [MI, 128, KH, 128], MM_DT, kind="ExternalInput")
    wd_d = nc.dram_tensor("wd", [128, KI, H], BF16, kind="ExternalInput")
    cw_d = nc.dram_tensor("cw", [128, CT], F32, kind="ExternalInput")
    xr_d = nc.dram_tensor("xr", [128, KH, TSH], F32, kind="ExternalInput")
    rw_d = nc.dram_tensor("rw", [128, KH, E], F32, kind="ExternalInput")
    y_d = nc.dram_tensor("y", [128, CT, H], F32, kind="ExternalOutput")
    aux_d = nc.dram_tensor("aux", [2 * E, 1], F32, kind="ExternalOutput")

    with tile.TileContext(nc) as tc, ExitStack() as ctx:
        const = ctx.enter_context(tc.tile_pool(name="const", bufs=1))
        xtp = ctx.enter_context(tc.tile_pool(name="xtp", bufs=1))
        htp = ctx.enter_context(tc.tile_pool(name="htp", bufs=1))
        wgp = ctx.enter_context(tc.tile_pool(name="wgp", bufs=5))
        wup = ctx.enter_context(tc.tile_pool(name="wup", bufs=5))
        wdp = ctx.enter_context(tc.tile_pool(name="wdp", bufs=1))
        sp = ctx.enter_context(tc.tile_pool(name="sp", bufs=3))
        yp = ctx.enter_context(tc.tile_pool(name="yp", bufs=4))
        rp = ctx.enter_context(tc.tile_pool(name="rp", bufs=2))
        ps = ctx.enter_context(tc.tile_pool(name="ps", bufs=8, space="PSUM"))

        # xt resident; block 0 + the first weight tiles go first so the PE
        # starts as early as possible
        xt_sb = xtp.tile([128, KH, C], MM_DT, tag="xt")
        wgus = []
        for m in range(MI):
            wg_sb = wgp.tile([128, KH, 128], MM_DT, tag="wg", name=f"wg{m}")
            wu_sb = wup.tile([128, KH, 128], MM_DT, tag="wu", name=f"wu{m}")
            if m == 0:
                kh2 = KH // 2
                nc.sync.dma_start(wg_sb[:, 0:kh2, :], wg_d[0, :, 0:kh2, :])
                nc.sync.dma_start(xt_sb[:, 0:kh2, 0:NB], xt_d[0][:, 0:kh2, :])
                nc.sync.dma_start(wg_sb[:, kh2:KH, :], wg_d[0, :, kh2:KH, :])
                nc.sync.dma_start(xt_sb[:, kh2:KH, 0:NB], xt_d[0][:, kh2:KH, :])
                nc.sync.dma_start(wu_sb[:, 0:kh2, :], wu_d[0, :, 0:kh2, :])
                nc.sync.dma_start(wu_sb[:, kh2:KH, :], wu_d[0, :, kh2:KH, :])
            else:
                nc.sync.dma_start(wg_sb[:], wg_d[m])
                nc.sync.dma_start(wu_sb[:], wu_d[m])
            wgus.append((wg_sb, wu_sb))
            if m == 0:
                for blk in range(1, NBLK):
                    nc.sync.dma_start(
                        xt_sb[:, :, blk * NB:(blk + 1) * NB],
                        xt_d[blk],
                    )
        cw_sb = const.tile([128, CT], F32)
        nc.sync.dma_start(cw_sb[:], cw_d[:])
        # wd is small in bf16: keep it resident, loaded during phase 1
        wd_sb = wdp.tile([128, KI, H], BF16, tag="wd")
        for i in range(KI):
            nc.sync.dma_start(wd_sb[:, i:i + 1, :], wd_d[:, i:i + 1, :])
        # prefetch router inputs early too (small)
        rw_sb = const.tile([128, KH, E], F32)
        nc.sync.dma_start(rw_sb[:], rw_d[:])
        xr_sb = const.tile([128, KH, TSH], F32)
        nc.sync.dma_start(xr_sb[:], xr_d[:])

        # ---- phase 1: gate/up + silu*up, weights loaded once (m-outer) ----
        ht = htp.tile([128, MI, C], BF16, tag="ht")
        for m in range(MI):
            wg_sb, wu_sb = wgus[m]
            for blk in range(NBLK):
                cs = slice(blk * NB, (blk + 1) * NB)
                psg = ps.tile([128, NB], F32, tag="ps", name=f"psg_{m}_{blk}")
                psu = ps.tile([128, NB], F32, tag="ps", name=f"psu_{m}_{blk}")
                for k in range(KH):
                    nc.tensor.matmul(
                        psg, wg_sb[:, k, :], xt_sb[:, k, cs],
                        start=(k == 0), stop=(k == KH - 1),
                    )
                for k in range(KH):
                    nc.tensor.matmul(
                        psu, wu_sb[:, k, :], xt_sb[:, k, cs],
                        start=(k == 0), stop=(k == KH - 1),
                    )
                s_sb = sp.tile([128, NB], F32, tag="s")
                nc.scalar.activation(s_sb, psg, AF.Silu)
                nc.vector.tensor_tensor(ht[:, m, cs], s_sb, psu, ALU.mult)

        # ---- router shard stats (logits + softmax stats), vectorized ----
        # (emitted between the phases: its vector work overlaps the down proj)
        ones_sb = const.tile([128, 1], F32)
        nc.any.memset(ones_sb[:], 1.0)
        NT = TSH // 128
        ps_r = ps.tile([128, NT, E], F32, tag="ps", name="psr")
        for t in range(NT):
            for k in range(KH):
                nc.tensor.matmul(
                    ps_r[:, t, :], xr_sb[:, k, t * 128:(t + 1) * 128],
                    rw_sb[:, k, :],
                    start=(t == 0 and k == 0), stop=(t == NT - 1 and k == KH - 1),
                    skip_group_check=True,
                )
        # logits ~ N(0,1): exp() cannot overflow fp32, skip the max-shift
        cat = rp.tile([128, NT, 2 * E], F32, tag="cat")
        ex = rp.tile([128, NT, E], F32, tag="ex")
        ssum = rp.tile([128, NT], F32, tag="ssum")
        rec = rp.tile([128, NT], F32, tag="rec")
        m1 = rp.tile([128, NT], F32, tag="m1")
        m2 = rp.tile([128, NT], F32, tag="m2")
        p2 = rp.tile([128, NT, E], F32, tag="p2")
        nc.scalar.activation(ex[:], ps_r, AF.Exp)
        nc.vector.tensor_reduce(ssum[:], ex[:], mybir.AxisListType.X, ALU.add)
        nc.vector.reciprocal(rec[:], ssum[:])
        nc.vector.tensor_tensor(
            cat[:, :, E:2 * E], ex[:],
            rec[:, :, None].to_broadcast((128, NT, E)), ALU.mult)
        nc.vector.tensor_reduce(m1[:], ex[:], mybir.AxisListType.X, ALU.max)
        nc.vector.tensor_tensor(
            p2[:], ex[:], m1[:, :, None].to_broadcast((128, NT, E)), ALU.is_equal)
        nc.vector.scalar_tensor_tensor(p2[:], p2[:], -1e6, ex[:], ALU.mult, ALU.add)
        nc.vector.tensor_reduce(m2[:], p2[:], mybir.AxisListType.X, ALU.max)
        nc.vector.tensor_tensor(
            cat[:, :, 0:E], ex[:],
            m2[:, :, None].to_broadcast((128, NT, E)), ALU.is_ge)

        # ---- phase 2: down proj, psum held over the full I contraction ----
        HS = H // 512
        for ct0 in range(0, CT, 2):
            nct = min(2, CT - ct0)
            psy = [
                [ps.tile([128, 512], F32, tag="ps", name=f"psy_{ct0}_{c}_{h}")
                 for h in range(HS)]
                for c in range(nct)
            ]
            for i in range(KI):
                for c in range(nct):
                    ct = ct0 + c
                    for hh in range(HS):
                        nc.tensor.matmul(
                            psy[c][hh],
                            ht[:, i, ct * 128:(ct + 1) * 128],
                            wd_sb[:, i, hh * 512:(hh + 1) * 512],
                            start=(i == 0), stop=(i == KI - 1),
                        )
            for c in range(nct):
                ct = ct0 + c
                for hh in range(HS):
                    y_sb = yp.tile([128, 512], F32, tag="y")
                    nc.vector.tensor_scalar_mul(
                        y_sb, psy[c][hh], cw_sb[:, ct:ct + 1]
                    )
                    nc.sync.dma_start(
                        y_d[:, ct, hh * 512:(hh + 1) * 512], y_sb
                    )

        # ---- tail: reduce the router stats across the token shard ----
        ps_aux = ps.tile([2 * E, 1], F32, tag="ps", name="psaux")
        for t in range(NT):
            nc.tensor.matmul(
                ps_aux, cat[:, t, :], ones_sb[:],
                start=(t == 0), stop=(t == NT - 1),
            )
        aux_sb = rp.tile([2 * E, 1], F32, tag="auxsb")
        nc.vector.tensor_copy(aux_sb[:], ps_aux)
        nc.sync.dma_start(aux_d[:], aux_sb[:])

    nc.compile()
    return nc


def _part_major(a, p=128):
    """[K*p, ...] -> [p, K, ...] with k = K_outer*p + p_inner on the first axis."""
    ko = a.shape[0] // p
    return np.ascontiguousarray(
        a.reshape(ko, p, *a.shape[1:]).transpose(1, 0, *range(2, a.ndim + 1))
    )


def kernel(hidden_states, router_w, w_gate, w_up, w_down):
    global LAST_RESULTS
    hidden_states = np.asarray(hidden_states, np.float32)
    router_w = np.ascontiguousarray(np.asarray(router_w, np.float32))
    w_gate = np.asarray(w_gate, np.float32)
    w_up = np.asarray(w_up, np.float32)
    w_down = np.asarray(w_down, np.float32)
    x = np.ascontiguousarray(hidden_states.reshape(T, H))

    # ---- host routing (for the dispatch indices + combine weights) ----
    logits = x @ router_w
    lmax = logits.max(-1, keepdims=True)
    p = np.exp(logits - lmax)
    p /= p.sum(-1, keepdims=True)
    rows = np.arange(T)
    i1 = p.argmax(-1)
    pm = p.copy()
    pm[rows, i1] = -1.0
    i2 = pm.argmax(-1)
    wsum = p[rows, i1] + p[rows, i2]

    idxs, cws = [], []
    for e in range(E):
        idx = np.nonzero((i1 == e) | (i2 == e))[0]
        idxs.append(idx)
        cws.append((p[idx, e] / wsum[idx]).astype(np.float32))
    maxn = max(len(ix) for ix in idxs)
    C = max(NB, ((maxn + NB - 1) // NB) * NB)
    CT = C // 128

    if C not in _BUILD_CACHE:
        _BUILD_CACHE[C] = _build(C)
    nc = _BUILD_CACHE[C]

    rw_t = _part_major(router_w)                      # [128, 8, 8]
    in_maps = []
    for e in range(E):
        idx = idxs[e]
        n = len(idx)
        xt = np.zeros((128, KH, C), np.float32)
        xt[:, :, :n] = _part_major(x[idx].T.copy())   # [128, 8, n]
        xt = np.ascontiguousarray(
            xt.reshape(128, KH, C // NB, NB).transpose(2, 0, 1, 3))
        cw = np.zeros(C, np.float32)
        cw[:n] = cws[e]
        cw = np.ascontiguousarray(cw.reshape(CT, 128).T)
        xr = _part_major(np.ascontiguousarray(x[e * TSH:(e + 1) * TSH].T))
        in_maps.append({
            "xt": xt,
            "wg": np.ascontiguousarray(
                w_gate[e].reshape(KH, 128, MI, 128).transpose(2, 1, 0, 3)),
            "wu": np.ascontiguousarray(
                w_up[e].reshape(KH, 128, MI, 128).transpose(2, 1, 0, 3)),
            "wd": _part_major(w_down[e].astype(ml_dtypes.bfloat16)),
            "cw": cw,
            "xr": xr,
            "rw": rw_t,
        })

    # trace defaults off; the test harness turns it on via BASS_TRACE=1
    res = run_bass_kernel_spmd(nc, in_maps, core_ids=list(range(NCORES)))
    LAST_RESULTS = res

    # ---- combine (scatter-add) + aux reduction ----
    out = np.zeros((T, H), np.float32)
    counts = np.zeros(E, np.float64)
    probsum = np.zeros(E, np.float64)
    for e in range(E):
        y = res.results[e]["y"]                       # [128, CT, H]
        y = y.transpose(1, 0, 2).reshape(C, H)
        idx = idxs[e]
        out[idx] += y[:len(idx)]
        aux = res.results[e]["aux"].reshape(2 * E)
        counts += aux[:E]
        probsum += aux[E:]
    aux_loss = np.float32(((counts / T) * (probsum / T)).sum() * E * AUX_COEF)
    return out.reshape(B, S, H), aux_loss

